# revision 20
# baseline (speedup 1.0000x reference)
"""TRN2 Bass kernel: ClapAudio window self-attention (B=2048 windows of 64
tokens, C=256, 8 heads x d=32), data-parallel over windows across 8 cores.

Host side: shards + pre-transposes hidden_states (xt [C, ntok]), precomputes
EB = exp(rel-pos-bias + mask)^T (folding both additive score biases into one
multiplicative table applied after exp), passes transposed weights in bf16.
Output returned bf16 from device, cast to f32 on host.

Device side (per core, 256 windows, 32 chunks of 8 windows), software
pipelined one chunk deep:  scores(u) -> [prep u+1: proj/qbd/V/va] -> ctx(u).
  - qbd: block-diagonal Q operand, band-contiguous layout (hh, g, w, q):
    band r of Q^T lands in col block hh=r via a trivial [32, 1024]
    contiguous SB->SB DMA; static zeros elsewhere. Scores take 16
    matmuls/chunk of [K=128, M=64, N=256] (4 heads per matmul, strided rhs).
  - exp via ACT (scale folded); EB multiply on gpsimd.
  - V projection; bv fused into the PSUM->SBUF cast (tensor_tensor add);
    V scattered into block-diag augmented va (ones cols for softmax sums)
    by 2 DVE copies + 2 partition-swapping strided SB->SB DMAs.
  - ctx: 8 matmuls/pair [K=128, M=64, N=66] pairing heads (j, j+4);
    DVE reciprocal + broadcast multiply normalizes into bf16 staging;
    1 output DMA per chunk.
"""

import numpy as np
import ml_dtypes

import concourse.bass as bass
import concourse.mybir as mybir
import concourse.tile as tile
from concourse.bass_utils import run_bass_kernel_spmd

DT = mybir.dt
F32 = DT.float32
BF16 = DT.bfloat16

N_CORES = 8
B = 2048
C = 256
H = 8
D = 32
WINTOK = 64
SCALE = 1.0 / np.sqrt(np.float32(D))


def _wait_cap(inst):
    """Max sem waits the walrus encoding of this instruction tolerates."""
    if isinstance(inst, (mybir.InstDrain, mybir.InstNoOp)):
        return 1  # CTRL_NO_STRUCT
    if isinstance(inst, (mybir.InstDMACopy, mybir.InstDMA, mybir.InstDmaTransposeAnt)):
        return 1  # PSEUDO_DMA_DIRECT2D
    return 1


def split_drain_waits(nc):
    """Walrus instruction encodings only fit a limited number of sem waits;
    Tile can attach more. Hoist excess waits onto NoOps inserted before the
    instruction on the same engine (in-order sequencers make this
    equivalent, if slightly more conservative)."""
    for f in nc.m.functions:
        for bb in f.blocks:
            new_insts = []
            for inst in bb.instructions:
                si = inst.sync_info
                cap = _wait_cap(inst)
                if si is not None and si.on_wait and len(si.on_wait) > cap:
                    waits = list(si.on_wait)
                    keep, rest = waits[:cap], waits[cap:]
                    for i in range(0, len(rest), 1):
                        new_insts.append(
                            mybir.InstNoOp(
                                name=f"{inst.name}-waitsplit-{i}",
                                engine=inst.engine,
                                sync_info=mybir.SyncInfo(
                                    on_wait=[rest[i]], on_update=[]
                                ),
                            )
                        )
                    inst.sync_info = mybir.SyncInfo(
                        on_wait=keep, on_update=list(si.on_update or [])
                    )
                new_insts.append(inst)
            bb.instructions[:] = new_insts


def build(n_windows=256, chunk_windows=8, split_waits=True):
    """Emit the per-core kernel.

    DRAM layouts:
      xt   [256, ntok] bf16 (host-transposed hidden states)
      eb   [32, 128, 512] bf16:
           eb[t, g*64+k, win*256+hh*64+q] =
               exp(rpb[g*4+hh, q, k] + mask[(2t+win)%64, q, k])
      wqt/wkt/wvt [256, 256] bf16 = W.T  ([C_in, C_out])
      bqv/bkv DRAM [256] f32 ; bvr DRAM [128, 256] bf16 (bv broadcast)
      out  [ntok, 256] bf16
    """
    assert n_windows % chunk_windows == 0 and chunk_windows % 2 == 0
    ntok = n_windows * WINTOK
    n_chunks = n_windows // chunk_windows
    chunk_tok = chunk_windows * WINTOK  # 512
    n_pairs = chunk_windows // 2  # 4 window pairs per chunk
    assert chunk_tok == 512

    nc = bass.Bass()
    xt = nc.declare_dram_parameter("xt", [C, ntok], BF16, isOutput=False)
    eb = nc.declare_dram_parameter("eb", [32, 128, 512], BF16, isOutput=False)
    wqt = nc.declare_dram_parameter("wqt", [C, C], BF16, isOutput=False)
    wkt = nc.declare_dram_parameter("wkt", [C, C], BF16, isOutput=False)
    wvt = nc.declare_dram_parameter("wvt", [C, C], BF16, isOutput=False)
    bqv = nc.declare_dram_parameter("bqv", [C], F32, isOutput=False)
    bkv = nc.declare_dram_parameter("bkv", [C], F32, isOutput=False)
    bvr = nc.declare_dram_parameter("bvr", [128, C], BF16, isOutput=False)
    out = nc.declare_dram_parameter("out", [ntok, C], BF16, isOutput=True)

    with tile.TileContext(nc) as tc:
        with (
            tc.tile_pool(name="const", bufs=1) as cpool,
            tc.tile_pool(name="acts", bufs=2) as apool,
            tc.tile_pool(name="probs", bufs=5) as ppool,
            tc.tile_pool(name="stage", bufs=2) as spool,
            tc.tile_pool(name="small", bufs=4) as smpool,
            tc.tile_pool(name="vv", bufs=2) as vpool,
            tc.tile_pool(name="ppj", bufs=2, space="PSUM") as ppj,
            tc.tile_pool(name="ppv", bufs=1, space="PSUM") as ppv,
            tc.tile_pool(name="psc", bufs=3, space="PSUM") as psc,
            tc.tile_pool(name="pctx", bufs=2, space="PSUM") as pctx,
        ):
            # ---- constants ----
            eb_sb = cpool.tile([128, 32 * 512], BF16)
            nc.sync.dma_start(
                eb_sb.rearrange("p (t c) -> p t c", t=32),
                eb.rearrange("t p c -> p t c"),
            )
            wq_sb = cpool.tile([128, 512], BF16)
            wk_sb = cpool.tile([128, 512], BF16)
            wv_sb = cpool.tile([128, 512], BF16)
            for w_sb, w_dram in ((wq_sb, wqt), (wk_sb, wkt), (wv_sb, wvt)):
                nc.sync.dma_start(
                    w_sb.rearrange("p (ck c) -> p ck c", ck=2),
                    w_dram.rearrange("(ck p) c -> p ck c", p=128),
                )
            bq_sb = cpool.tile([128, 2], F32)
            bk_sb = cpool.tile([128, 2], F32)
            nc.sync.dma_start(bq_sb[:], bqv.rearrange("(m p) -> p m", p=128))
            nc.sync.dma_start(bk_sb[:], bkv.rearrange("(m p) -> p m", p=128))
            bvb_sb = cpool.tile([128, C], BF16)
            nc.sync.dma_start(bvb_sb[:], bvr[:, :])

            # qbd: block-diag Q operand, band-contiguous (hh, g, w, q);
            # double buffered, zeros static.
            qbds = []
            for ub in range(2):
                qb = cpool.tile([128, 4 * 1024], BF16, tag=f"qbd{ub}")
                nc.vector.memset(qb[:], 0.0)
                qbds.append(qb)

            # va: per chunk [128, (win, i, j, 66)]; block (win, i, j):
            # rows 0:64  cols +0:32  = V_{j}(w)[k, d],   col +32 ones
            # rows 64:128 cols +33:65 = V_{j+4}(w)[k, d], col +65 ones
            # (win-major layout so swap-DMA dest APs collapse to 3 dims)
            vas = []
            for ub in range(2):
                va = cpool.tile([128, 8 * 264], BF16, tag=f"vaall{ub}")
                nc.vector.memset(va[:], 0.0)
                nc.vector.memset(
                    va[0:64, :].rearrange("p (w j c) -> p w j c", w=8, j=4)[
                        :, :, :, 32:33
                    ],
                    1.0,
                )
                nc.vector.memset(
                    va[64:128, :].rearrange("p (w j c) -> p w j c", w=8, j=4)[
                        :, :, :, 65:66
                    ],
                    1.0,
                )
                vas.append(va)

            def load_xt(u):
                xt_sb = apool.tile([128, 2 * 512], BF16, tag="xt")
                t0 = u * chunk_tok
                nc.sync.dma_start(
                    xt_sb.rearrange("p (ck t) -> p ck t", ck=2),
                    xt.rearrange("(ck p) t -> p ck t", p=128)[
                        :, :, t0 : t0 + chunk_tok
                    ],
                )
                return xt_sb

            def prep_qk(u, xt_sb):
                """Q/K projections + qbd for chunk u."""
                qbd = qbds[u % 2]

                qt_sb = apool.tile([128, 2 * 512], BF16, tag="qt")
                kt_sb = apool.tile([128, 2 * 512], BF16, tag="kt")
                for qk, (w_sb, b_sb, dst) in enumerate(
                    ((wq_sb, bq_sb, qt_sb), (wk_sb, bk_sb, kt_sb))
                ):
                    for m in range(2):
                        prj = ppj.tile([128, 512], F32, tag="ppj")
                        for ck in range(2):
                            nc.tensor.matmul(
                                prj[:],
                                w_sb[:, ck * 256 + m * 128 : ck * 256 + (m + 1) * 128],
                                xt_sb[:, ck * 512 : (ck + 1) * 512],
                                start=(ck == 0),
                                stop=(ck == 1),
                            )
                        cp_out = dst[:, m * 512 : (m + 1) * 512]
                        if qk == 0:
                            nc.scalar.activation(
                                cp_out,
                                prj[:],
                                mybir.ActivationFunctionType.Identity,
                                bias=b_sb[:, m : m + 1],
                            )
                        else:
                            nc.vector.tensor_scalar_add(
                                cp_out, prj[:], b_sb[:, m : m + 1]
                            )

                # qbd band DMAs: contiguous [32, 1024] each; 2 on the sync
                # HWDGE ring, 2 on the scalar ring.
                for r in range(4):
                    eng = nc.sync if r % 2 == 0 else nc.scalar
                    eng.dma_start(
                        qbd[32 * r : 32 * r + 32, r * 1024 : (r + 1) * 1024],
                        qt_sb[32 * r : 32 * r + 32, :],
                    )
                return kt_sb, qbd

            def prep_v(u, xt_sb):
                """V projection + va assembly for chunk u."""
                va = vas[u % 2]
                # V projection; bv fused into the cast. vtmp cols are
                # (m, i, j, d) so each swap-DMA source is one contiguous run.
                vtmp = vpool.tile([128, 4 * 256], BF16, tag="vt")
                for i in range(4):
                    vps = ppv.tile([128, 256], F32, tag="ppv")
                    for ck in range(2):
                        nc.tensor.matmul(
                            vps[:],
                            xt_sb[:, ck * 512 + i * 128 : ck * 512 + (i + 1) * 128],
                            wv_sb[:, ck * 256 : (ck + 1) * 256],
                            start=(ck == 0),
                            stop=(ck == 1),
                        )
                    nc.vector.tensor_tensor(
                        vtmp.rearrange("p (m i jd) -> p m i jd", m=2, i=4)[:, :, i, :],
                        vps.rearrange("p (m jd) -> p m jd", m=2),
                        bvb_sb.rearrange("p (m jd) -> p m jd", m=2),
                        mybir.AluOpType.add,
                    )

                # va cols: (win, i, j, 66). vtmp cols: (m, i, j, d).
                va_top = va[0:64, :].rearrange(
                    "p (win i j c) -> p win i j c", win=2, i=4, j=4
                )
                va_bot = va[64:128, :].rearrange(
                    "p (win i j c) -> p win i j c", win=2, i=4, j=4
                )
                # partition-swapping fills via SB->SB DMA (3-dim APs)
                nc.sync.dma_start(
                    va_top[:, 1, :, :, 0:32], vtmp[64:128, 0:512]
                )
                nc.sync.dma_start(
                    va_bot[:, 0, :, :, 33:65], vtmp[0:64, 512:1024]
                )
                # same-partition fills on gpsimd (SBUF->SBUF)
                nc.gpsimd.tensor_copy(
                    va_top[:, 0, :, :, 0:32],
                    vtmp[0:64, 0:512].rearrange("p (i j d) -> p i j d", i=4, j=4),
                )
                nc.gpsimd.tensor_copy(
                    va_bot[:, 1, :, :, 33:65],
                    vtmp[64:128, 512:1024].rearrange("p (i j d) -> p i j d", i=4, j=4),
                )
                return va

            def scores(u, kt_sb, qbd):
                """Score matmuls + exp + EB multiply; returns probs tiles."""
                qbd_v = qbd.rearrange("p (hh g w q) -> p hh g w q", hh=4, g=2, w=8)
                probs_l = []
                for i in range(n_pairs):
                    scp = psc.tile([128, 512], F32, tag="sc")
                    for g in range(2):
                        for win in range(2):
                            wl = i * 2 + win
                            nc.tensor.matmul(
                                scp[g * 64 : g * 64 + 64, win * 256 : win * 256 + 256],
                                kt_sb[:, g * 512 + wl * 64 : g * 512 + wl * 64 + 64],
                                qbd_v[:, :, g, wl, :],
                                start=True,
                                stop=True,
                                tile_position=(0, g * 64),
                            )
                    probs = ppool.tile([128, 512], BF16, tag="pr")
                    nc.scalar.activation(
                        probs[:],
                        scp[:],
                        mybir.ActivationFunctionType.Exp,
                        scale=float(SCALE),
                    )
                    t_slot = (u * n_pairs + i) % 32
                    ebs = eb_sb[:, t_slot * 512 : (t_slot + 1) * 512]
                    if i == 1:
                        nc.vector.tensor_mul(probs[:], probs[:], ebs)
                    else:
                        nc.gpsimd.tensor_mul(probs[:], probs[:], ebs)
                    probs_l.append(probs)
                return probs_l

            def ctx_out(u, probs_l, va):
                stgc = spool.tile([128, 4 * 256], BF16, tag="st")
                for i in range(n_pairs):
                    probs = probs_l[i]
                    ctxp = pctx.tile([128, 264], F32, tag="ctx")
                    for win in range(2):
                        for j in range(4):
                            nc.tensor.matmul(
                                ctxp[win * 64 : win * 64 + 64, j * 66 : j * 66 + 66],
                                probs[:, win * 256 + j * 64 : win * 256 + j * 64 + 64],
                                va[
                                    :,
                                    win * 1056 + i * 264 + j * 66 : win * 1056
                                    + i * 264
                                    + j * 66
                                    + 66,
                                ],
                                start=True,
                                stop=True,
                                tile_position=(0, win * 64),
                            )
                    recips = smpool.tile([128, 8], F32, tag="rc")
                    sums_ap = ctxp.rearrange("p (j par c) -> p j par c", j=4, par=2)[
                        :, :, :, 32:33
                    ]
                    nc.vector.reciprocal(recips[:], sums_ap)
                    ctx_ap = ctxp.rearrange("p (j par c) -> p par j c", j=4, par=2)[
                        :, :, :, 0:32
                    ]
                    rec_ap = recips.rearrange(
                        "p (j par one) -> p par j one", j=4, par=2, one=1
                    )
                    ctx_b, rec_b = bass.broadcast_tensor_aps(ctx_ap, rec_ap)
                    out_ap = stgc[:, i * 256 : (i + 1) * 256].rearrange(
                        "p (par j c) -> p par j c", par=2, j=4
                    )
                    nc.vector.tensor_tensor(out_ap, ctx_b, rec_b, mybir.AluOpType.mult)
                t0 = u * chunk_tok
                nc.sync.dma_start(
                    out[t0 : t0 + chunk_tok, :].rearrange("(i p) c -> p i c", p=128),
                    stgc.rearrange("p (i c) -> p i c", i=4),
                )

            # ---- software-pipelined main loop ----
            # per-iteration engine order: projQK(u+1) -> scores(u) ->
            # projV(u+1) -> ctx(u); keeps the PE queue fed while exp/ebmul
            # latency drains, and gets qbd(u+1) built a full chunk early.
            xt_cur = load_xt(0)
            kt_sb, qbd = prep_qk(0, xt_cur)
            va = prep_v(0, xt_cur)
            for u in range(n_chunks):
                if u + 1 < n_chunks:
                    xt_nxt = load_xt(u + 1)
                    kq_nxt = prep_qk(u + 1, xt_nxt)
                probs_l = scores(u, kt_sb, qbd)
                if u + 1 < n_chunks:
                    va_nxt = prep_v(u + 1, xt_nxt)
                ctx_out(u, probs_l, va)
                if u + 1 < n_chunks:
                    kt_sb, qbd = kq_nxt
                    va = va_nxt

    if split_waits:
        split_drain_waits(nc)
    return nc


_NC_CACHE = {}


def _get_nc():
    key = "main"
    if key not in _NC_CACHE:
        _NC_CACHE[key] = build(n_windows=B // N_CORES)
    return _NC_CACHE[key]


def _pack_eb(bias_table, rel_index, attention_mask):
    # rpb[h, q, k] = bias_table[rel_index[q, k], h]
    rpb = bias_table[rel_index.reshape(-1)].reshape(64, 64, H).transpose(2, 0, 1)
    e = np.exp(
        rpb[None].astype(np.float64) + attention_mask[:, None].astype(np.float64)
    ).astype(np.float32)
    # e [nw, h, q, k] -> eb[t, g*64 + k, win*256 + hh*64 + q]
    # where nw = 2t + win, h = g*4 + hh
    e2 = e.transpose(0, 1, 3, 2)  # [nw, h, k, q]
    e3 = e2.reshape(32, 2, 2, 4, 64, 64)  # [t, win, g, hh, k, q]
    e4 = e3.transpose(0, 2, 4, 1, 3, 5)  # [t, g, k, win, hh, q]
    return np.ascontiguousarray(e4.reshape(32, 128, 512))


def build_in_maps(
    hidden_states,
    attention_mask,
    Wq,
    bq,
    Wk,
    bk,
    Wv,
    bv,
    bias_table,
    rel_index,
):
    bf = ml_dtypes.bfloat16
    xs = np.ascontiguousarray(
        np.asarray(hidden_states, np.float32).reshape(B * WINTOK, C).T
    ).astype(bf)
    eb = _pack_eb(
        np.asarray(bias_table, np.float32),
        np.asarray(rel_index),
        np.asarray(attention_mask, np.float32),
    ).astype(bf)
    common = {
        "eb": eb,
        "wqt": np.ascontiguousarray(Wq.T).astype(bf),
        "wkt": np.ascontiguousarray(Wk.T).astype(bf),
        "wvt": np.ascontiguousarray(Wv.T).astype(bf),
        "bqv": np.asarray(bq, np.float32),
        "bkv": np.asarray(bk, np.float32),
        "bvr": np.tile(np.asarray(bv, np.float32)[None, :], (128, 1)).astype(bf),
    }
    shard_tok = (B // N_CORES) * WINTOK
    return [
        {"xt": np.ascontiguousarray(xs[:, c * shard_tok : (c + 1) * shard_tok]), **common}
        for c in range(N_CORES)
    ]


def kernel(
    hidden_states,
    attention_mask,
    Wq,
    bq,
    Wk,
    bk,
    Wv,
    bv,
    bias_table,
    rel_index,
):
    nc = _get_nc()
    in_maps = build_in_maps(
        hidden_states, attention_mask, Wq, bq, Wk, bk, Wv, bv, bias_table, rel_index
    )
    res = run_bass_kernel_spmd(nc, in_maps, list(range(N_CORES)))
    outp = np.concatenate(
        [res.results[c]["out"] for c in range(N_CORES)], axis=0
    )
    return outp.reshape(B, WINTOK, C).astype(np.float32)
